# revision 9
# baseline (speedup 1.0000x reference)
"""Trainium2 Bass kernel: fused multi-head attention block (projections +
softmax attention + output projection + residual + LayerNorm).

Sharding: 8 cores = 2 batches x 4 token-chunks of 512. Each core projects
K/V/Q only for its OWN 512 tokens, then the K^T and V tensors are
AllGather-ed across the 4-core batch group (DRAM bounce buffers). Each
core then runs full attention for its 512 queries over all 2048 keys,
the output projection, residual add and LayerNorm.

Device-side layouts (per core):
  xt   [1024, 512]  bf16  x[b]^T own-token columns (feature-major)
  xq   [512, 1024]  f32   own-token rows of x[b] (residual input)
  wq/wk/wv [1024, 1024] bf16  [c, h*64+d] (head-minor)
  wo   [1024, 1024] bf16  [(h*64+d), m]
  bias [16, 128]    f32   additive key bias per key tile/partition
                          (mask bias - EXP_SHIFT)
  gamma/beta [1024] bf16
Output: y [512, 1024] f32.
"""

import contextlib

import numpy as np
import ml_dtypes

import concourse.bass as bass
import concourse.tile as tile
from concourse import mybir
from concourse import bass_utils

BF16 = ml_dtypes.bfloat16
N_CORES = 8
B, L, D, H, DH = 2, 2048, 1024, 16, 64
Q = L // 4          # tokens owned per core
CT = D // 128       # contraction tiles over features
JT = L // 128       # key tiles
IT = Q // 128       # query tiles per core
LN_EPS = 1e-5
GROUPS = [[0, 1, 2, 3], [4, 5, 6, 7]]

PV_FP8 = False      # fp8e4m3 P/V with DoubleRow matmuls in the PV stage
EXP_SHIFT = 7.0 if PV_FP8 else 0.0

F32 = mybir.dt.float32
BF = mybir.dt.bfloat16
E8 = mybir.dt.float8e4
PVDT = E8 if PV_FP8 else BF


def _split_waits(nc, maxw=1):
    """This walrus build rejects instructions with more than one sync wait;
    split excess waits into preceding NOPs on the same engine."""
    ctr = 0
    for fn in nc.m.functions:
        for bb in fn.blocks:
            new_insts = []
            for inst in bb.instructions:
                si = inst.sync_info
                if si is not None and len(si.on_wait) > maxw:
                    waits = list(si.on_wait)
                    excess, keep = waits[:-maxw], waits[-maxw:]
                    for i in range(0, len(excess), maxw):
                        ctr += 1
                        new_insts.append(mybir.InstNoOp(
                            name=f"waitsplit_nop_{ctr}",
                            engine=inst.engine,
                            sync_info=mybir.SyncInfo(
                                on_wait=excess[i:i + maxw], on_update=[]),
                            text_hint="waitsplit",
                        ))
                    si.on_wait = keep
                new_insts.append(inst)
            bb.instructions = new_insts
    return ctr


def _emit(nc, tc, hh, masked):
    Exp = mybir.ActivationFunctionType.Exp
    Sqrt = mybir.ActivationFunctionType.Sqrt
    DR = mybir.MatmulPerfMode.DoubleRow

    xt_ap = hh["xt"].ap().rearrange("(t p) q -> p t q", p=128)   # [128,8,512]
    wq_ap = hh["wq"].ap().rearrange("(t p) d -> p t d", p=128)
    wk_ap = hh["wk"].ap().rearrange("(t p) d -> p t d", p=128)
    wv_ap = hh["wv"].ap().rearrange("(t p) d -> p t d", p=128)
    wo_ap = hh["wo"].ap().rearrange("(t p) d -> p t d", p=128)
    bias_ap = hh["bias"].ap().rearrange("a b -> b a")            # [128,16]
    xq_ap = hh["xq"].ap().rearrange("(t p) d -> p t d", p=128)   # [128,4,1024]
    y_ap = hh["y"].ap()

    def bcast_dram(h1d, parts=128):
        a = h1d.ap()
        return bass.AP(tensor=a.tensor, offset=a.offset,
                       ap=[[0, parts]] + list(a.ap))

    with contextlib.ExitStack() as ctx:
        dram = ctx.enter_context(tc.tile_pool(name="dram", bufs=1,
                                              space="DRAM"))
        const = ctx.enter_context(tc.tile_pool(name="const", bufs=1))
        wpool = ctx.enter_context(tc.tile_pool(name="wpool", bufs=2))
        vctp = ctx.enter_context(tc.tile_pool(name="vctp", bufs=1))
        qtp = ctx.enter_context(tc.tile_pool(name="qtp", bufs=1))
        ktp = ctx.enter_context(tc.tile_pool(name="ktp", bufs=2))
        vp = ctx.enter_context(tc.tile_pool(name="vp", bufs=2))
        expp = ctx.enter_context(tc.tile_pool(name="expp", bufs=2))
        ptp = ctx.enter_context(tc.tile_pool(name="ptp", bufs=1))
        npool = ctx.enter_context(tc.tile_pool(name="npool", bufs=4))
        xqp = ctx.enter_context(tc.tile_pool(name="xqp", bufs=1))
        lnp = ctx.enter_context(tc.tile_pool(name="lnp", bufs=3))
        statp = ctx.enter_context(tc.tile_pool(name="statp", bufs=4))
        psS = ctx.enter_context(tc.tile_pool(name="psS", bufs=2, space="PSUM"))
        psP = ctx.enter_context(tc.tile_pool(name="psP", bufs=2, space="PSUM"))
        psB = ctx.enter_context(tc.tile_pool(name="psB", bufs=2, space="PSUM"))

        # DRAM bounce buffers for the gathers
        kc = dram.tile([D, Q], BF)             # own K^T chunk [d', own keys]
        kg = dram.tile([4, D, Q], BF)          # gathered K^T
        vc = dram.tile([Q, H * (DH + 1)], PVDT)  # own V chunk (+ones cols)
        vg = dram.tile([4, Q, H * (DH + 1)], PVDT)

        # ---- constants / small loads ----
        eps_sb = const.tile([128, 1], F32)
        nc.vector.memset(eps_sb[:], LN_EPS)
        ones64 = const.tile([1, 64], BF)
        nc.vector.memset(ones64[:], 1.0)
        bias_sb = const.tile([128, 16], F32)
        gamma_sb = const.tile([128, 1024], BF)
        beta_sb = const.tile([128, 1024], BF)

        # ---- input streams: xt on SP queue, weights on ACT queue ----
        xt_sb = const.tile([128, CT, Q], BF)
        nc.sync.dma_start(out=xt_sb[:], in_=xt_ap)
        wk_sb = wpool.tile([128, CT, 1024], BF, tag="w")
        nc.scalar.dma_start(out=wk_sb[:, 0:4, :], in_=wk_ap[:, 0:4, :])
        nc.scalar.dma_start(out=wk_sb[:, 4:8, :], in_=wk_ap[:, 4:8, :])

        # ---- K^T projection (own keys): [d' 128][keys 512] ----
        # staged in the probt tile (dead until phase B; kc DMA orders reuse)
        probt = ptp.tile([128, 8, Q], BF)
        kct = probt
        for dtp in range(4):
            ps = psS.tile([128, 2, 512], F32, tag="ss")
            for half in range(2):
                dt = 2 * dtp + half
                for ct in range(CT):
                    nc.tensor.matmul(
                        ps[:, half, :],
                        wk_sb[:, ct, dt * 128:(dt + 1) * 128],
                        xt_sb[:, ct, :],
                        start=(ct == 0), stop=(ct == CT - 1))
            nc.vector.tensor_copy(kct[:, 2 * dtp:2 * dtp + 2, :], ps[:])
        nc.sync.dma_start(
            out=kc[:].rearrange("(t p) q -> p t q", p=128), in_=kct[:])
        nc.gpsimd.collective_compute(
            "AllGather", mybir.AluOpType.bypass, replica_groups=GROUPS,
            ins=[kc[:].opt()], outs=[kg[:].opt()])

        wv_sb = wpool.tile([128, CT, 1024], BF, tag="w")
        nc.scalar.dma_start(out=wv_sb[:], in_=wv_ap)

        # ---- V projection (own tokens): [token 128][h*64] + ones col ----
        vct = vctp.tile([128, 4, H, DH + 1], PVDT)
        nc.vector.memset(vct[:, :, :, DH:DH + 1], 1.0)
        for lt in range(4):
            ps = psS.tile([128, 2, 512], F32, tag="ss")
            for nt in range(2):
                for ct in range(CT):
                    nc.tensor.matmul(
                        ps[:, nt, :],
                        xt_sb[:, ct, lt * 128:(lt + 1) * 128],
                        wv_sb[:, ct, nt * 512:(nt + 1) * 512],
                        start=(ct == 0), stop=(ct == CT - 1))
            nc.vector.tensor_copy(
                vct[:, lt, :, 0:DH],
                ps.rearrange("p n (h d) -> p (n h) d", h=8))
        nc.sync.dma_start(
            out=vc[:].rearrange("(t p) f -> p t f", p=128), in_=vct[:])
        nc.gpsimd.collective_compute(
            "AllGather", mybir.AluOpType.bypass, replica_groups=GROUPS,
            ins=[vc[:].opt()], outs=[vg[:].opt()])

        wq_sb = wpool.tile([128, CT, 1024], BF, tag="w")
        nc.scalar.dma_start(out=wq_sb[:], in_=wq_ap)

        # ---- Q^T projection: [d' 128][queries 512] ----
        qt_all = qtp.tile([128, 8, Q], BF)
        for dtp in range(4):
            ps = psS.tile([128, 2, 512], F32, tag="ss")
            for half in range(2):
                dt = 2 * dtp + half
                for ct in range(CT):
                    nc.tensor.matmul(
                        ps[:, half, :],
                        wq_sb[:, ct, dt * 128:(dt + 1) * 128],
                        xt_sb[:, ct, :],
                        start=(ct == 0), stop=(ct == CT - 1))
            nc.vector.tensor_copy(qt_all[:, 2 * dtp:2 * dtp + 2, :], ps[:])

        # remaining loads for phases B/C
        wo_sb = wpool.tile([128, CT, 1024], BF, tag="w")
        nc.scalar.dma_start(out=wo_sb[:], in_=wo_ap)
        nc.scalar.dma_start(out=bias_sb[:], in_=bias_ap)
        nc.scalar.dma_start(out=gamma_sb[:], in_=bcast_dram(hh["gamma"]))
        nc.scalar.dma_start(out=beta_sb[:], in_=bcast_dram(hh["beta"]))
        xq_sb = xqp.tile([128, IT, 1024], F32)
        nc.sync.dma_start(out=xq_sb[:], in_=xq_ap)

        # ---- attention loop over d'-tiles (= head pairs) ----
        for dt in range(8):
            kt_t = ktp.tile([128, 4, Q], BF, tag="kt")
            nc.sync.dma_start(
                out=kt_t[:],
                in_=kg[:, dt * 128:(dt + 1) * 128, :]
                .rearrange("c p q -> p c q"))
            v_t = vp.tile([128, JT, 2, DH + 1], PVDT, tag="v")
            nc.gpsimd.dma_start(
                out=v_t[:],
                in_=vg[:, :, 2 * dt * (DH + 1):(2 * dt + 2) * (DH + 1)]
                .rearrange("c (t p) f -> p (c t) f", p=128))
            expt = expp.tile([128, JT, 2, 512], PVDT, tag="e")
            pv_ps = [psP.tile([DH + 1, 512], F32, tag="pp", name=f"pv{hb}")
                     for hb in range(2)]
            for jt in range(JT):
                ps = psS.tile([128, 2, 512], F32, tag="ss")
                for hb in range(2):
                    nc.tensor.matmul(
                        ps[:, hb, :],
                        kt_t[hb * 64:hb * 64 + 64, jt // 4,
                             (jt % 4) * 128:(jt % 4) * 128 + 128],
                        qt_all[hb * 64:hb * 64 + 64, dt, :],
                        start=True, stop=True)
                if masked:
                    nc.scalar.activation(
                        expt[:, jt, :, :], ps[:], Exp,
                        bias=bias_sb[:, jt:jt + 1], scale=1.0 / 8.0)
                else:
                    nc.scalar.activation(
                        expt[:, jt, :, :], ps[:], Exp,
                        bias=-EXP_SHIFT, scale=1.0 / 8.0)
                if PV_FP8:
                    if jt % 2 == 1:
                        jp = jt // 2
                        for hb in range(2):
                            nc.tensor.matmul(
                                pv_ps[hb][:],
                                v_t[:, 2 * jp:2 * jp + 2, hb, 0:DH + 1],
                                expt[:, 2 * jp:2 * jp + 2, hb, :],
                                start=(jp == 0), stop=(jp == JT // 2 - 1),
                                perf_mode=DR)
                else:
                    for hb in range(2):
                        nc.tensor.matmul(
                            pv_ps[hb][:], v_t[:, jt, hb, 0:DH + 1],
                            expt[:, jt, hb, :],
                            start=(jt == 0), stop=(jt == JT - 1))
            for hb in range(2):
                rdiv = npool.tile([1, 512], BF, tag="n")
                with nc.allow_low_precision(reason="bf16 softmax denom"):
                    nc.vector.reciprocal(rdiv[:], pv_ps[hb][DH:DH + 1, :])
                ps_b = psB.tile([64, 512], F32, tag="bb")
                nc.tensor.matmul(ps_b[:], ones64[:], rdiv[:],
                                 start=True, stop=True)
                rdivb = npool.tile([64, 512], F32, tag="nb")
                nc.vector.tensor_copy(rdivb[:], ps_b[:])
                nc.vector.tensor_mul(
                    probt[hb * 64:hb * 64 + 64, dt, :],
                    pv_ps[hb][0:DH, :], rdivb[:])

        # ---- output projection + residual + LayerNorm ----
        for it in range(IT):
            ps_r = psS.tile([128, 2, 512], F32, tag="ss")
            for mh in range(2):
                for kt in range(8):
                    nc.tensor.matmul(
                        ps_r[:, mh, :],
                        probt[:, kt, it * 128:(it + 1) * 128],
                        wo_sb[:, kt, mh * 512:(mh + 1) * 512],
                        start=(kt == 0), stop=(kt == 7))
            h_sb = lnp.tile([128, 1024], F32, tag="ln")
            nc.vector.tensor_add(h_sb[:], ps_r.rearrange("p a b -> p (a b)"),
                                 xq_sb[:, it, :])
            stats = statp.tile([128, 2, 6], F32)
            nc.vector.bn_stats(stats[:, 0, :], h_sb[:, 0:512])
            nc.vector.bn_stats(stats[:, 1, :], h_sb[:, 512:1024])
            mv = statp.tile([128, 2], F32)
            nc.vector.bn_aggr(mv[:], stats[:])
            std = statp.tile([128, 1], F32)
            nc.scalar.activation(std[:], mv[:, 1:2], Sqrt,
                                 bias=eps_sb[:], scale=1.0)
            rstd = statp.tile([128, 1], F32)
            nc.vector.reciprocal(rstd[:], std[:])
            t1 = lnp.tile([128, 1024], F32, tag="ln")
            nc.vector.tensor_scalar(
                t1[:], h_sb[:], mv[:, 0:1], rstd[:],
                op0=mybir.AluOpType.subtract, op1=mybir.AluOpType.mult)
            t2 = lnp.tile([128, 1024], F32, tag="ln")
            nc.vector.tensor_mul(t2[:], t1[:], gamma_sb[:])
            out_t = lnp.tile([128, 1024], F32, tag="ln")
            nc.vector.tensor_add(out_t[:], t2[:], beta_sb[:])
            nc.sync.dma_start(y_ap[it * 128:(it + 1) * 128, :], out_t[:])


def build_module(split=True, masked=False):
    nc = bass.Bass("TRN2", target_bir_lowering=False, debug=False,
                   num_devices=N_CORES)
    hh = {
        "xt": nc.dram_tensor("xt", [D, Q], BF, kind="ExternalInput"),
        "xq": nc.dram_tensor("xq", [Q, D], F32, kind="ExternalInput"),
        "wq": nc.dram_tensor("wq", [D, D], BF, kind="ExternalInput"),
        "wk": nc.dram_tensor("wk", [D, D], BF, kind="ExternalInput"),
        "wv": nc.dram_tensor("wv", [D, D], BF, kind="ExternalInput"),
        "wo": nc.dram_tensor("wo", [D, D], BF, kind="ExternalInput"),
        "bias": nc.dram_tensor("bias", [16, 128], F32, kind="ExternalInput"),
        "gamma": nc.dram_tensor("gamma", [D], BF, kind="ExternalInput"),
        "beta": nc.dram_tensor("beta", [D], BF, kind="ExternalInput"),
        "y": nc.dram_tensor("y", [Q, D], F32, kind="ExternalOutput"),
    }
    with tile.TileContext(nc) as tc:
        _emit(nc, tc, hh, masked)
    if split:
        _split_waits(nc, 1)
    return nc


_CACHE = {}


def get_module(masked=False):
    key = ("nc", masked)
    if key not in _CACHE:
        _CACHE[key] = build_module(masked=masked)
    return _CACHE[key]


def prep_inputs(x, mask, w_q, w_k, w_v, w_o, ln_gamma, ln_beta):
    x = np.asarray(x, dtype=np.float32)
    mask = np.asarray(mask)
    shared = {
        "wq": np.ascontiguousarray(
            np.asarray(w_q, np.float32).transpose(1, 0, 2).reshape(D, D)
        ).astype(BF16),
        "wk": np.ascontiguousarray(
            np.asarray(w_k, np.float32).transpose(1, 0, 2).reshape(D, D)
        ).astype(BF16),
        "wv": np.ascontiguousarray(
            np.asarray(w_v, np.float32).transpose(1, 0, 2).reshape(D, D)
        ).astype(BF16),
        "wo": np.asarray(w_o, np.float32).reshape(D, D).astype(BF16),
        "gamma": np.asarray(ln_gamma, np.float32).astype(BF16),
        "beta": np.asarray(ln_beta, np.float32).astype(BF16),
    }
    in_maps = []
    for c in range(N_CORES):
        b, q0 = c // 4, (c % 4) * Q
        m = {
            "xt": np.ascontiguousarray(
                x[b].T[:, q0:q0 + Q]).astype(BF16),
            "xq": np.ascontiguousarray(x[b, q0:q0 + Q, :]),
            "bias": (np.where(mask[b], 0.0, -1e9) - EXP_SHIFT).astype(
                np.float32).reshape(16, 128),
        }
        m.update(shared)
        in_maps.append(m)
    masked = not bool(mask.all())
    return in_maps, masked


def assemble(results):
    out = np.empty((B, L, D), dtype=np.float32)
    for c in range(N_CORES):
        b, q0 = c // 4, (c % 4) * Q
        out[b, q0:q0 + Q, :] = results[c]["y"]
    return out


def run(in_maps, masked=False, **kwargs):
    nc = get_module(masked)
    return bass_utils.run_bass_kernel_spmd(
        nc, in_maps, core_ids=list(range(N_CORES)), **kwargs)


def kernel(x, mask, w_q, w_k, w_v, w_o, ln_gamma, ln_beta):
    in_maps, masked = prep_inputs(x, mask, w_q, w_k, w_v, w_o,
                                  ln_gamma, ln_beta)
    res = run(in_maps, masked)
    return assemble(res.results)
